# revision 35
# baseline (speedup 1.0000x reference)
"""CenterLoss kernel for Trainium2 (8 NeuronCores, Bass/Tile).

Problem (fixed shapes):
    features [32768, 512] f32, labels [32768] int64 in [0, 1000), centers [1000, 512] f32
    loss        = 0.5 * sum((features - centers[labels])^2) / B
    new_centers = where(count>0, (1-a)*centers + a*(segsum(features)/count), centers), a=0.5

Strategy
--------
Shard by CLASS RANGE: core j owns classes [125j, 125j+125). Host routes each
batch row to the core owning its label (this is the shard step — a host-side
fancy-index, same as any data-parallel slice). Each core then scatter-adds its
~4096 rows into ONE 128-partition accumulator tile via a one-hot matmul
(onehot[b, c].T @ feat[b, d], accumulated in PSUM across row tiles), so the PE
does 8x less work than a 1000-class one-hot and no cross-core reduction of the
[1000, 512] state is needed -- classes are owned exclusively.

The gather (centers[labels]) is eliminated algebraically:
    sum_i ||f_i - c_{l_i}||^2 = sum||f||^2 - 2*<segsum(f), centers> + sum_c count_c*||c_c||^2
All three terms come from the segment sums/counts we need anyway.

Everything on device is fp32 (matmul operands bitcast to float32r: exact for
the 0/1 one-hot weights; moving operand passes through the PE fp32 path at
1 cycle/row for N>=256). PSUM accumulation is fp32.

Per-core engine budget (T = 36 row-tiles of 128):
    DMA  ~9 MB in (features) -> ~25 us  <- bound (target_regime=memory)
    PE   36 x (512-cycle sums MM + 1-col counts MM) ~ 10-14 us
    DVE  36 x 128-cycle one-hot is_equal + epilogue ~ 8 us
    ACT  36 x 512-cycle square+row-accum (for sum||f||^2) ~ 13 us
"""

import math
import os

import numpy as np

import concourse.bass as bass
import concourse.bacc as bacc
import concourse.mybir as mybir
import concourse.tile as tile
from concourse.bass_utils import run_bass_kernel_spmd

NUM_CLASSES = 1000
FEAT_DIM = 512
ALPHA = 0.5
N_CORES = 8
CPC = NUM_CLASSES // N_CORES  # 125 classes per core
P = 128
G = 6  # 128-row subtiles per DMA group (1.5 MiB loads)

# matmul operand mode:
#   "f32r" - PE fast fp32 path (1 cyc/row at N>=256), DMA stays 4B/elem
#   "f32"  - exact fp32, 4 cyc/row on PE
#   "bf16" - operands in bf16: halves feature DMA traffic; PSUM accum fp32
MM_DTYPE = os.environ.get("CENTERLOSS_MM_DTYPE", "f32r")
# "raw" = hand-scheduled engine blocks; "tile" = TileContext version
IMPL = os.environ.get("CENTERLOSS_IMPL", "raw")

_NC_CACHE: dict = {}


def _mm_dt():
    return {
        "f32r": mybir.dt.float32r,
        "f32": mybir.dt.float32,
        "bf16": mybir.dt.bfloat16,
    }[MM_DTYPE]


def _np_feat_dt():
    import ml_dtypes

    return ml_dtypes.bfloat16 if MM_DTYPE == "bf16" else np.float32


def _group_sizes(T: int):
    """Small ramp-up/taper groups cut pipeline fill/drain latency."""
    if T <= 6:
        return [min(T, 2), T - 2] if T > 2 else [T]
    rest = T - 12
    return [2, 4] + [6] * (rest // 6) + [4, 2]


def _pack_features(fj, T):
    """Pack [Bp, D] rows into the device layout: one [P, w*D] block per group
    (subtile u of group g -> block cols [u*D, (u+1)*D)), zero-padded to G*D."""
    gs = _group_sizes(T)
    nG = len(gs)
    out = np.zeros((nG * P, G * FEAT_DIM), fj.dtype)
    off = 0
    for k, w in enumerate(gs):
        blk = fj[off * P : (off + w) * P].reshape(w, P, FEAT_DIM)
        out[k * P : (k + 1) * P, : w * FEAT_DIM] = (
            blk.transpose(1, 0, 2).reshape(P, w * FEAT_DIM)
        )
        off += w
    return out


def _build_raw(T: int) -> bass.Bass:
    """Raw-bacc pipeline (no TileContext): explicit engine programs and
    semaphores. Avoids Tile's ~3us start barrier and ~13us drain butterfly.

    Engines: Sync issues DMAs (triple-buffered feature groups), Vector builds
    the wide one-hot + counts accumulator + epilogue blend, Tensor runs the
    PSUM-accumulated one-hot matmuls, Scalar does square+row-accumulate.
    """
    f32 = mybir.dt.float32
    mdt = _mm_dt()
    gs = _group_sizes(T)
    nG = len(gs)
    offs = [sum(gs[:k]) for k in range(nG)]
    FS = 3  # ft slots
    OS = 2  # oh slots
    nc = bacc.Bacc(None, target_bir_lowering=False)

    feat = nc.declare_dram_parameter("feat", [nG * P, G * FEAT_DIM], mdt, isOutput=False)
    lab2d = nc.declare_dram_parameter("lab2d", [P, T], f32, isOutput=False)
    cent = nc.declare_dram_parameter("cent", [P, FEAT_DIM], f32, isOutput=False)
    iota_in = nc.declare_dram_parameter("iota", [P, P], f32, isOutput=False)
    out_c = nc.declare_dram_parameter("out_centers", [P, FEAT_DIM], f32, isOutput=True)
    out_p = nc.declare_dram_parameter("out_partials", [P, 4], f32, isOutput=True)

    feat_v = feat[:].rearrange("(g p) m -> g p m", p=P)

    from contextlib import ExitStack

    ctx = ExitStack()
    with ctx:
        ft = ctx.enter_context(nc.sbuf_tensor("ft", [P, FS, G * FEAT_DIM], mdt))
        oh = ctx.enter_context(nc.sbuf_tensor("oh", [P, OS, G * P], mdt))
        iota_f = ctx.enter_context(nc.sbuf_tensor("iota_f", [P, P], f32))
        lab_s = ctx.enter_context(nc.sbuf_tensor("lab_s", [P, T], f32))
        cent_s = ctx.enter_context(nc.sbuf_tensor("cent_s", [P, FEAT_DIM], f32))
        counts_acc = ctx.enter_context(nc.sbuf_tensor("counts_acc", [P, G * P], f32))
        ssq_cols = ctx.enter_context(nc.sbuf_tensor("ssq_cols", [P, nG], f32))
        sq_scr = ctx.enter_context(nc.sbuf_tensor("sq_scr", [P, 2, G * FEAT_DIM], f32))
        ones_s = ctx.enter_context(nc.sbuf_tensor("ones_s", [P, 2], mdt))
        ones_f = ctx.enter_context(nc.sbuf_tensor("ones_f", [P, 2], f32))
        counts_pc = ctx.enter_context(nc.sbuf_tensor("counts_pc", [P, P], f32))
        counts_pc_m = ctx.enter_context(nc.sbuf_tensor("counts_pc_m", [P, P], mdt))
        counts_f = ctx.enter_context(nc.sbuf_tensor("counts_f", [P, 1], f32))
        recip = ctx.enter_context(nc.sbuf_tensor("recip", [P, 1], f32))
        mask = ctx.enter_context(nc.sbuf_tensor("mask", [P, 1], f32))
        mean = ctx.enter_context(nc.sbuf_tensor("mean", [P, FEAT_DIM], f32))
        diff = ctx.enter_context(nc.sbuf_tensor("diff", [P, FEAT_DIM], f32))
        newc = ctx.enter_context(nc.sbuf_tensor("newc", [P, FEAT_DIM], f32))
        scr = ctx.enter_context(nc.sbuf_tensor("scr", [P, FEAT_DIM], f32))
        csq = ctx.enter_context(nc.sbuf_tensor("csq", [P, 1], f32))
        partials = ctx.enter_context(nc.sbuf_tensor("partials", [P, 4], f32))
        sums_ps = ctx.enter_context(nc.psum_tensor("sums_ps", [P, FEAT_DIM], f32))
        counts_ps = ctx.enter_context(nc.psum_tensor("counts_ps", [P, 2], f32))
        dma_const = ctx.enter_context(nc.semaphore("dma_const"))
        dma_ft = [
            ctx.enter_context(nc.semaphore(f"dma_ft{i}")) for i in range(FS)
        ]
        dve_oh = ctx.enter_context(nc.semaphore("dve_oh"))
        pe_grp = ctx.enter_context(nc.semaphore("pe_grp"))
        act_grp = ctx.enter_context(nc.semaphore("act_grp"))
        dve_fin = ctx.enter_context(nc.semaphore("dve_fin"))
        pe_fin = ctx.enter_context(nc.semaphore("pe_fin"))
        dma_out = ctx.enter_context(nc.semaphore("dma_out"))
        block = ctx.enter_context(nc.Block())
        iota_b = bass.AP(
            tensor=iota_f, offset=0,
            ap=[iota_f.ap().ap[0], [0, G], iota_f.ap().ap[1]],
        )  # [P, G, P]
        acc_cu = bass.AP(
            tensor=counts_acc, offset=0,
            ap=[counts_acc.ap().ap[0], [1, P], [P, G]],
        )  # [P, c, u]

        @block.sync
        def _(sync):
            # first feature group leads; tiny constants right behind it
            sync.dma_start(
                out=ft[:, 0, : gs[0] * FEAT_DIM],
                in_=feat_v[0][:, : gs[0] * FEAT_DIM],
            ).then_inc(dma_ft[0], 16)
            sync.dma_start(out=lab_s[:], in_=lab2d[:]).then_inc(dma_const, 16)
            sync.dma_start(out=iota_f[:], in_=iota_in[:]).then_inc(dma_const, 16)
            sync.dma_start(out=cent_s[:], in_=cent[:]).then_inc(dma_const, 16)
            for g in range(1, nG):
                if g >= FS:
                    # slot free when group g-FS fully consumed by PE and ACT
                    sync.wait_ge(pe_grp, g - FS + 1)
                    sync.wait_ge(act_grp, g - FS + 1)
                sync.dma_start(
                    out=ft[:, g % FS, : gs[g] * FEAT_DIM],
                    in_=feat_v[g][:, : gs[g] * FEAT_DIM],
                ).then_inc(dma_ft[g % FS], 16)
            # outputs
            sync.wait_ge(dve_fin, 2)
            sync.dma_start(out=out_c[:], in_=newc[:]).then_inc(dma_out, 16)
            sync.wait_ge(dve_fin, 3)
            sync.dma_start(out=out_p[:], in_=partials[:]).then_inc(dma_out, 16)
            sync.wait_ge(dma_out, 32)

        @block.vector
        def _(vector):
            vector.memset(counts_acc[:], 0.0)
            vector.memset(ones_f[:], 1.0)
            vector.drain()
            vector.tensor_copy(ones_s[:], ones_f[:])
            vector.drain()
            vector.wait_ge(dma_const, 48)
            for g in range(nG):
                if g >= OS:
                    vector.wait_ge(pe_grp, g - OS + 1)
                w = gs[g]
                lab_b = lab_s[:, offs[g] : offs[g] + w].to_broadcast([P, w, P])
                iota_bg = bass.AP(
                    tensor=iota_f, offset=0,
                    ap=[iota_f.ap().ap[0], [0, w], iota_f.ap().ap[1]],
                )
                vector.tensor_tensor(
                    out=oh[:, g % OS, : w * P].rearrange("p (u c) -> p u c", u=w),
                    in0=iota_bg, in1=lab_b, op=mybir.AluOpType.is_equal,
                ).then_inc(dve_oh, 1)
                vector.wait_ge(dve_oh, g + 1)
                vector.tensor_tensor(
                    out=counts_acc[:, : w * P], in0=counts_acc[:, : w * P],
                    in1=oh[:, g % OS, : w * P],
                    op=mybir.AluOpType.add,
                )
            # counts: reduce u, convert for the final matmul
            vector.drain()
            vector.tensor_reduce(
                counts_pc[:], acc_cu, axis=mybir.AxisListType.X,
                op=mybir.AluOpType.add,
            )
            vector.drain()
            vector.tensor_copy(counts_pc_m[:], counts_pc[:])
            vector.drain().then_inc(dve_fin, 1)
            # blend (needs final sums + counts matmul)
            vector.wait_ge(pe_grp, nG)
            vector.wait_ge(pe_fin, 1)
            vector.tensor_copy(counts_f[:], counts_ps[:, 0:1])
            vector.drain()
            vector.tensor_scalar_max(recip[:], counts_f[:], 1.0)
            vector.tensor_scalar(
                mask[:], counts_f[:], 0.0, None, mybir.AluOpType.is_gt
            )
            vector.drain()
            vector.reciprocal(recip[:], recip[:])
            vector.drain()
            # newc = cent*(1 - a*m) + sums * (recip*m*a)
            vector.tensor_scalar(
                diff[:, 0:1], recip[:], mask[:, :1], ALPHA,
                mybir.AluOpType.mult, mybir.AluOpType.mult,
            )
            vector.tensor_scalar(
                diff[:, 1:2], mask[:], -ALPHA, 1.0,
                mybir.AluOpType.mult, mybir.AluOpType.add,
            )
            vector.drain()
            vector.tensor_scalar_mul(newc[:], cent_s[:], diff[:, 1:2])
            vector.tensor_scalar_mul(mean[:], sums_ps[:], diff[:, 0:1])
            vector.drain()
            vector.tensor_tensor(
                out=newc[:], in0=newc[:], in1=mean[:], op=mybir.AluOpType.add
            )
            vector.drain().then_inc(dve_fin, 1)
            # loss partials
            vector.wait_ge(act_grp, nG + 1)  # all squares + csq done
            vector.tensor_reduce(
                partials[:, 0:1], ssq_cols[:], axis=mybir.AxisListType.X,
                op=mybir.AluOpType.add,
            )
            vector.tensor_tensor(
                out=scr[:], in0=sums_ps[:], in1=cent_s[:], op=mybir.AluOpType.mult
            )
            vector.drain()
            vector.tensor_reduce(
                partials[:, 1:2], scr[:], axis=mybir.AxisListType.X,
                op=mybir.AluOpType.add,
            )
            vector.tensor_tensor(
                out=partials[:, 2:3], in0=csq[:], in1=counts_f[:],
                op=mybir.AluOpType.mult,
            )
            vector.memset(partials[:, 3:4], 0.0)
            vector.drain()
            vector.nop().then_inc(dve_fin, 1)

        @block.tensor
        def _(tensor):
            for g in range(nG):
                tensor.wait_ge(dma_ft[g % FS], 16 * (g // FS + 1))
                tensor.wait_ge(dve_oh, g + 1)
                for u in range(gs[g]):
                    t = offs[g] + u
                    mm = nc.tensor.matmul(
                        sums_ps[:],
                        lhsT=oh[:, g % OS, u * P : (u + 1) * P],
                        rhs=ft[:, g % FS, u * FEAT_DIM : (u + 1) * FEAT_DIM],
                        start=(t == 0), stop=(t == T - 1),
                    )
                    if u == gs[g] - 1:
                        mm.then_inc(pe_grp, 1)
            tensor.wait_ge(dve_fin, 1)
            nc.tensor.matmul(
                counts_ps[:], lhsT=counts_pc_m[:], rhs=ones_s[:],
                start=True, stop=True,
            ).then_inc(pe_fin, 1)

        @block.scalar
        def _(scalar):
            for g in range(nG):
                scalar.wait_ge(dma_ft[g % FS], 16 * (g // FS + 1))
                nc.scalar.activation(
                    sq_scr[:, g % 2, : gs[g] * FEAT_DIM],
                    ft[:, g % FS, : gs[g] * FEAT_DIM],
                    mybir.ActivationFunctionType.Square,
                    accum_out=ssq_cols[:, g : g + 1],
                ).then_inc(act_grp, 1)
            scalar.wait_ge(dma_const, 48)
            scalar.drain()
            nc.scalar.activation(
                sq_scr[:, 1, :FEAT_DIM], cent_s[:],
                mybir.ActivationFunctionType.Square,
                accum_out=csq[:],
            )
            scalar.drain().then_inc(act_grp, 1)

    nc.compile()
    return nc


def _build(T: int) -> bass.Bass:
    """One SPMD program; all 8 cores run it on their own shard."""
    f32 = mybir.dt.float32
    mdt = _mm_dt()
    nG = T // G
    nc = bacc.Bacc(None, target_bir_lowering=False)

    feat = nc.declare_dram_parameter("feat", [nG * P, G * FEAT_DIM], mdt, isOutput=False)
    lab2d = nc.declare_dram_parameter("lab2d", [P, T], f32, isOutput=False)
    cent = nc.declare_dram_parameter("cent", [P, FEAT_DIM], f32, isOutput=False)
    out_c = nc.declare_dram_parameter("out_centers", [P, FEAT_DIM], f32, isOutput=True)
    out_p = nc.declare_dram_parameter("out_partials", [P, 4], f32, isOutput=True)

    with tile.TileContext(nc) as tc:
        with (
            tc.tile_pool(name="singles", bufs=1) as singles,
            tc.tile_pool(name="ftiles", bufs=3) as ftiles,
            tc.tile_pool(name="ohpool", bufs=4) as ohpool,
            tc.tile_pool(name="sqpool", bufs=2) as sqpool,
            tc.tile_pool(name="psum", bufs=1, space="PSUM") as psum,
        ):
            # constants / whole-kernel inputs
            iota_i = singles.tile([P, P], mybir.dt.int32)
            nc.gpsimd.iota(iota_i[:], [[1, P]], channel_multiplier=0)
            iota_f = singles.tile([P, P], f32)
            nc.vector.tensor_copy(iota_f[:], iota_i[:])

            lab_s = singles.tile([P, T], f32)
            nc.sync.dma_start(out=lab_s[:], in_=lab2d[:])
            cent_s = singles.tile([P, FEAT_DIM], f32)
            nc.sync.dma_start(out=cent_s[:], in_=cent[:])
            # N=2 keeps the counts matmul legal under fp32r (even free count)
            ones_f = singles.tile([P, 2], f32)
            nc.vector.memset(ones_f[:], 1.0)
            ones_s = singles.tile([P, 2], mdt)
            nc.vector.tensor_copy(ones_s[:], ones_f[:])
            ssq_cols = singles.tile([P, nG], f32)

            # per-(partition, u, class) one-hot accumulator; reduced to counts
            # at the end (replaces a per-subtile counts matmul)
            counts_acc = singles.tile([P, G * P], f32)
            nc.vector.memset(counts_acc[:], 0.0)

            sums_ps = psum.tile([P, FEAT_DIM], f32)   # one full PSUM bank
            counts_ps = psum.tile([P, 2], f32)        # its own bank (padded)

            feat_v = feat[:].rearrange("(g p) m -> g p m", p=P)
            iota_ap = iota_f[:]
            iota_b = bass.AP(
                tensor=iota_ap.tensor, offset=iota_ap.offset,
                ap=[iota_ap.ap[0], [0, G], iota_ap.ap[1]],
            )  # [P, G, P], same iota row for every u
            for g in range(nG):
                ft = ftiles.tile([P, G, FEAT_DIM], mdt)
                nc.sync.dma_start(out=ft[:], in_=feat_v[g])
                # one wide one-hot for the whole group: oh[p, u, c] = (c == lab[g*G+u][p])
                oh = ohpool.tile([P, G, P], mdt)
                lab_b = lab_s[:, g * G : (g + 1) * G].to_broadcast([P, G, P])
                nc.vector.tensor_tensor(
                    out=oh[:], in0=iota_b, in1=lab_b, op=mybir.AluOpType.is_equal
                )
                nc.vector.tensor_tensor(
                    out=counts_acc[:], in0=counts_acc[:], in1=oh[:].rearrange("p u c -> p (u c)"),
                    op=mybir.AluOpType.add,
                )
                for u in range(G):
                    t = g * G + u
                    nc.tensor.matmul(
                        sums_ps[:], lhsT=oh[:, u, :], rhs=ft[:, u, :],
                        start=(t == 0), stop=(t == T - 1),
                    )
                sq = sqpool.tile([P, G, FEAT_DIM], f32)
                nc.scalar.activation(
                    sq[:], ft[:], mybir.ActivationFunctionType.Square,
                    accum_out=ssq_cols[:, g : g + 1],
                )

            # ---- epilogue: center blend + loss partials ----
            # counts[c] = sum_{p,u} counts_acc[p, u*P + c]: reduce u on DVE,
            # then one matmul with a ones rhs for the partition reduction.
            acc_ap = counts_acc[:]
            acc_cu = bass.AP(
                tensor=acc_ap.tensor, offset=acc_ap.offset,
                ap=[acc_ap.ap[0], [1, P], [P, G]],
            )  # [P, c, u] with u innermost
            counts_pc = singles.tile([P, P], f32)
            nc.vector.tensor_reduce(
                counts_pc[:], acc_cu, axis=mybir.AxisListType.X,
                op=mybir.AluOpType.add,
            )
            counts_pc_m = singles.tile([P, P], mdt)
            nc.vector.tensor_copy(counts_pc_m[:], counts_pc[:])
            nc.tensor.matmul(
                counts_ps[:], lhsT=counts_pc_m[:], rhs=ones_s[:],
                start=True, stop=True,
            )
            counts_f = singles.tile([P, 1], f32)
            nc.vector.tensor_copy(counts_f[:], counts_ps[:, 0:1])
            safe = singles.tile([P, 1], f32)
            nc.vector.tensor_scalar_max(safe[:], counts_f[:], 1.0)
            recip = singles.tile([P, 1], f32)
            nc.vector.reciprocal(recip[:], safe[:])
            mean = singles.tile([P, FEAT_DIM], f32)
            nc.vector.tensor_scalar_mul(mean[:], sums_ps[:], recip[:, :1])
            mask = singles.tile([P, 1], f32)
            nc.vector.tensor_scalar(
                mask[:], counts_f[:], 0.0, None, mybir.AluOpType.is_gt
            )
            diff = singles.tile([P, FEAT_DIM], f32)
            nc.vector.tensor_tensor(
                out=diff[:], in0=mean[:], in1=cent_s[:], op=mybir.AluOpType.subtract
            )
            dm = singles.tile([P, FEAT_DIM], f32)
            # dm = (diff * mask) * ALPHA
            nc.vector.tensor_scalar(
                dm[:], diff[:], mask[:, :1], ALPHA,
                mybir.AluOpType.mult, mybir.AluOpType.mult,
            )
            newc = singles.tile([P, FEAT_DIM], f32)
            nc.vector.tensor_tensor(
                out=newc[:], in0=cent_s[:], in1=dm[:], op=mybir.AluOpType.add
            )
            nc.sync.dma_start(out=out_c[:], in_=newc[:])

            partials = singles.tile([P, 4], f32)
            nc.vector.tensor_reduce(
                partials[:, 0:1], ssq_cols[:], axis=mybir.AxisListType.X,
                op=mybir.AluOpType.add,
            )
            # (tensor_tensor_reduce crashes the exec unit on this HW path;
            # use separate mul + reduce)
            scr = singles.tile([P, FEAT_DIM], f32)
            nc.vector.tensor_tensor(
                out=scr[:], in0=sums_ps[:], in1=cent_s[:], op=mybir.AluOpType.mult
            )
            nc.vector.tensor_reduce(
                partials[:, 1:2], scr[:], axis=mybir.AxisListType.X,
                op=mybir.AluOpType.add,
            )
            scr2 = singles.tile([P, FEAT_DIM], f32)
            csq = singles.tile([P, 1], f32)
            nc.scalar.activation(
                scr2[:], cent_s[:], mybir.ActivationFunctionType.Square,
                accum_out=csq[:],
            )
            nc.vector.tensor_tensor(
                out=partials[:, 2:3], in0=csq[:], in1=counts_f[:],
                op=mybir.AluOpType.mult,
            )
            nc.vector.memset(partials[:, 3:4], 0.0)
            nc.sync.dma_start(out=out_p[:], in_=partials[:])

    nc.compile()
    return nc


def _get_nc(T: int) -> bass.Bass:
    key = (IMPL, MM_DTYPE, T)
    if key not in _NC_CACHE:
        _NC_CACHE[key] = _build_raw(T) if IMPL == "raw" else _build(T)
    return _NC_CACHE[key]


def _shard_inputs(features, labels, centers):
    """Route rows to the core owning their class; build per-core maps."""
    order = np.argsort(labels, kind="stable")
    sl = labels[order]
    bounds = np.searchsorted(sl, np.arange(0, NUM_CLASSES + 1, CPC))
    n_max = int(np.diff(bounds).max())
    T = max(G, math.ceil(n_max / P))
    T = math.ceil(T / G) * G
    Bp = T * P
    nG = T // G

    fdt = _np_feat_dt()
    in_maps = []
    for j in range(N_CORES):
        idx = order[bounds[j] : bounds[j + 1]]
        n = len(idx)
        fj = np.zeros((Bp, FEAT_DIM), fdt)
        fj[:n] = features[idx].astype(fdt)
        lj = np.full(Bp, P - 1, np.float32)  # pad rows -> dummy class 127
        lj[:n] = (sl[bounds[j] : bounds[j + 1]] - j * CPC).astype(np.float32)
        fdev = (
            _pack_features(fj, T)
            if IMPL == "raw"
            else np.ascontiguousarray(
                fj.reshape(nG, G, P, FEAT_DIM).transpose(0, 2, 1, 3)
            ).reshape(nG * P, G * FEAT_DIM)
        )
        ldev = np.ascontiguousarray(lj.reshape(T, P).T)
        cj = np.zeros((P, FEAT_DIM), np.float32)
        cj[:CPC] = centers[j * CPC : (j + 1) * CPC]
        m = {"feat": fdev, "lab2d": ldev, "cent": cj}
        if IMPL == "raw":
            m["iota"] = np.broadcast_to(
                np.arange(P, dtype=np.float32), (P, P)
            ).copy()
        in_maps.append(m)
    return in_maps, T


def _run(features, labels, centers, trace=False):
    in_maps, T = _shard_inputs(features, labels, centers)
    nc = _get_nc(T)
    out = run_bass_kernel_spmd(
        nc, in_maps, core_ids=list(range(N_CORES)), trace=trace
    )
    res = out.results
    new_centers = np.empty((NUM_CLASSES, FEAT_DIM), np.float32)
    ssq = dot = cn = 0.0
    for j in range(N_CORES):
        new_centers[j * CPC : (j + 1) * CPC] = res[j]["out_centers"][:CPC]
        pr = np.asarray(res[j]["out_partials"], dtype=np.float64)
        ssq += pr[:, 0].sum()
        dot += pr[:, 1].sum()
        cn += pr[:, 2].sum()
    B = features.shape[0]
    loss = np.float32(0.5 * (ssq - 2.0 * dot + cn) / B)
    return (loss, new_centers), out


def kernel(features, labels, centers):
    features = np.ascontiguousarray(np.asarray(features), dtype=np.float32)
    labels = np.asarray(labels).astype(np.int64)
    centers = np.ascontiguousarray(np.asarray(centers), dtype=np.float32)
    (loss, new_centers), _ = _run(features, labels, centers, trace=False)
    return loss, new_centers


# revision 36
# speedup vs baseline: 1.3246x; 1.3246x over previous
"""CenterLoss kernel for Trainium2 (8 NeuronCores, Bass/Tile).

Problem (fixed shapes):
    features [32768, 512] f32, labels [32768] int64 in [0, 1000), centers [1000, 512] f32
    loss        = 0.5 * sum((features - centers[labels])^2) / B
    new_centers = where(count>0, (1-a)*centers + a*(segsum(features)/count), centers), a=0.5

Strategy
--------
Shard by CLASS RANGE: core j owns classes [125j, 125j+125). Host routes each
batch row to the core owning its label (this is the shard step — a host-side
fancy-index, same as any data-parallel slice). Each core then scatter-adds its
~4096 rows into ONE 128-partition accumulator tile via a one-hot matmul
(onehot[b, c].T @ feat[b, d], accumulated in PSUM across row tiles), so the PE
does 8x less work than a 1000-class one-hot and no cross-core reduction of the
[1000, 512] state is needed -- classes are owned exclusively.

The gather (centers[labels]) is eliminated algebraically:
    sum_i ||f_i - c_{l_i}||^2 = sum||f||^2 - 2*<segsum(f), centers> + sum_c count_c*||c_c||^2
All three terms come from the segment sums/counts we need anyway.

Everything on device is fp32 (matmul operands bitcast to float32r: exact for
the 0/1 one-hot weights; moving operand passes through the PE fp32 path at
1 cycle/row for N>=256). PSUM accumulation is fp32.

Per-core engine budget (T = 36 row-tiles of 128):
    DMA  ~9 MB in (features) -> ~25 us  <- bound (target_regime=memory)
    PE   36 x (512-cycle sums MM + 1-col counts MM) ~ 10-14 us
    DVE  36 x 128-cycle one-hot is_equal + epilogue ~ 8 us
    ACT  36 x 512-cycle square+row-accum (for sum||f||^2) ~ 13 us
"""

import math
import os

import numpy as np

import concourse.bass as bass
import concourse.bacc as bacc
import concourse.mybir as mybir
import concourse.tile as tile
from concourse.bass_utils import run_bass_kernel_spmd

NUM_CLASSES = 1000
FEAT_DIM = 512
ALPHA = 0.5
N_CORES = 8
CPC = NUM_CLASSES // N_CORES  # 125 classes per core
P = 128
G = 6  # 128-row subtiles per DMA group (1.5 MiB loads)

# matmul operand mode:
#   "f32r" - PE fast fp32 path (1 cyc/row at N>=256), DMA stays 4B/elem
#   "f32"  - exact fp32, 4 cyc/row on PE
#   "bf16" - operands in bf16: halves feature DMA traffic; PSUM accum fp32
MM_DTYPE = os.environ.get("CENTERLOSS_MM_DTYPE", "f32r")
# "raw" = hand-scheduled engine blocks; "tile" = TileContext version
IMPL = os.environ.get("CENTERLOSS_IMPL", "raw")

_NC_CACHE: dict = {}


def _mm_dt():
    return {
        "f32r": mybir.dt.float32r,
        "f32": mybir.dt.float32,
        "bf16": mybir.dt.bfloat16,
    }[MM_DTYPE]


def _np_feat_dt():
    import ml_dtypes

    return ml_dtypes.bfloat16 if MM_DTYPE == "bf16" else np.float32


def _group_sizes(T: int):
    return [G] * (T // G)


def _pack_features(fj, T):
    """Pack [Bp, D] rows into the device layout: one [P, w*D] block per group
    (subtile u of group g -> block cols [u*D, (u+1)*D)), zero-padded to G*D."""
    gs = _group_sizes(T)
    nG = len(gs)
    out = np.zeros((nG * P, G * FEAT_DIM), fj.dtype)
    off = 0
    for k, w in enumerate(gs):
        blk = fj[off * P : (off + w) * P].reshape(w, P, FEAT_DIM)
        out[k * P : (k + 1) * P, : w * FEAT_DIM] = (
            blk.transpose(1, 0, 2).reshape(P, w * FEAT_DIM)
        )
        off += w
    return out


def _build_raw(T: int) -> bass.Bass:
    """Raw-bacc pipeline (no TileContext): explicit engine programs and
    semaphores. Avoids Tile's ~3us start barrier and ~13us drain butterfly.

    Engines: Sync issues DMAs (triple-buffered feature groups), Vector builds
    the wide one-hot + counts accumulator + epilogue blend, Tensor runs the
    PSUM-accumulated one-hot matmuls, Scalar does square+row-accumulate.
    """
    f32 = mybir.dt.float32
    mdt = _mm_dt()
    gs = _group_sizes(T)
    nG = len(gs)
    offs = [sum(gs[:k]) for k in range(nG)]
    FS = 3  # ft slots
    OS = 2  # oh slots
    nc = bacc.Bacc(None, target_bir_lowering=False)

    feat = nc.declare_dram_parameter("feat", [nG * P, G * FEAT_DIM], mdt, isOutput=False)
    lab2d = nc.declare_dram_parameter("lab2d", [P, T], f32, isOutput=False)
    cent = nc.declare_dram_parameter("cent", [P, FEAT_DIM], f32, isOutput=False)
    iota_in = nc.declare_dram_parameter("iota", [P, P], f32, isOutput=False)
    out_c = nc.declare_dram_parameter("out_centers", [P, FEAT_DIM], f32, isOutput=True)
    out_p = nc.declare_dram_parameter("out_partials", [P, 4], f32, isOutput=True)

    feat_v = feat[:].rearrange("(g p) m -> g p m", p=P)

    from contextlib import ExitStack

    ctx = ExitStack()
    with ctx:
        ft = ctx.enter_context(nc.sbuf_tensor("ft", [P, FS, G * FEAT_DIM], mdt))
        oh = ctx.enter_context(nc.sbuf_tensor("oh", [P, OS, G * P], mdt))
        iota_f = ctx.enter_context(nc.sbuf_tensor("iota_f", [P, P], f32))
        lab_s = ctx.enter_context(nc.sbuf_tensor("lab_s", [P, T], f32))
        cent_s = ctx.enter_context(nc.sbuf_tensor("cent_s", [P, FEAT_DIM], f32))
        counts_acc = ctx.enter_context(nc.sbuf_tensor("counts_acc", [P, G * P], f32))
        ssq_cols = ctx.enter_context(nc.sbuf_tensor("ssq_cols", [P, nG], f32))
        sq_scr = ctx.enter_context(nc.sbuf_tensor("sq_scr", [P, 2, G * FEAT_DIM], f32))
        ones_s = ctx.enter_context(nc.sbuf_tensor("ones_s", [P, 2], mdt))
        ones_f = ctx.enter_context(nc.sbuf_tensor("ones_f", [P, 2], f32))
        counts_pc = ctx.enter_context(nc.sbuf_tensor("counts_pc", [P, P], f32))
        counts_pc_m = ctx.enter_context(nc.sbuf_tensor("counts_pc_m", [P, P], mdt))
        counts_f = ctx.enter_context(nc.sbuf_tensor("counts_f", [P, 1], f32))
        recip = ctx.enter_context(nc.sbuf_tensor("recip", [P, 1], f32))
        mask = ctx.enter_context(nc.sbuf_tensor("mask", [P, 1], f32))
        mean = ctx.enter_context(nc.sbuf_tensor("mean", [P, FEAT_DIM], f32))
        diff = ctx.enter_context(nc.sbuf_tensor("diff", [P, FEAT_DIM], f32))
        newc = ctx.enter_context(nc.sbuf_tensor("newc", [P, FEAT_DIM], f32))
        scr = ctx.enter_context(nc.sbuf_tensor("scr", [P, FEAT_DIM], f32))
        csq = ctx.enter_context(nc.sbuf_tensor("csq", [P, 1], f32))
        partials = ctx.enter_context(nc.sbuf_tensor("partials", [P, 4], f32))
        sums_ps = ctx.enter_context(nc.psum_tensor("sums_ps", [P, FEAT_DIM], f32))
        counts_ps = ctx.enter_context(nc.psum_tensor("counts_ps", [P, 2], f32))
        dma_const = ctx.enter_context(nc.semaphore("dma_const"))
        dma_ft = [
            ctx.enter_context(nc.semaphore(f"dma_ft{i}")) for i in range(FS)
        ]
        dve_oh = ctx.enter_context(nc.semaphore("dve_oh"))
        pe_grp = ctx.enter_context(nc.semaphore("pe_grp"))
        act_grp = ctx.enter_context(nc.semaphore("act_grp"))
        dve_fin = ctx.enter_context(nc.semaphore("dve_fin"))
        pe_fin = ctx.enter_context(nc.semaphore("pe_fin"))
        dma_out = ctx.enter_context(nc.semaphore("dma_out"))
        block = ctx.enter_context(nc.Block())
        iota_b = bass.AP(
            tensor=iota_f, offset=0,
            ap=[iota_f.ap().ap[0], [0, G], iota_f.ap().ap[1]],
        )  # [P, G, P]
        acc_cu = bass.AP(
            tensor=counts_acc, offset=0,
            ap=[counts_acc.ap().ap[0], [1, P], [P, G]],
        )  # [P, c, u]

        @block.sync
        def _(sync):
            # first feature group leads; tiny constants right behind it
            sync.dma_start(
                out=ft[:, 0, : gs[0] * FEAT_DIM],
                in_=feat_v[0][:, : gs[0] * FEAT_DIM],
            ).then_inc(dma_ft[0], 16)
            sync.dma_start(out=lab_s[:], in_=lab2d[:]).then_inc(dma_const, 16)
            sync.dma_start(out=iota_f[:], in_=iota_in[:]).then_inc(dma_const, 16)
            sync.dma_start(out=cent_s[:], in_=cent[:]).then_inc(dma_const, 16)
            # (ft0 was issued first above; remaining groups below)
            for g in range(1, nG):
                if g >= FS:
                    # slot free when group g-FS fully consumed by PE and ACT
                    sync.wait_ge(pe_grp, g - FS + 1)
                    sync.wait_ge(act_grp, g - FS + 2)
                sync.dma_start(
                    out=ft[:, g % FS, : gs[g] * FEAT_DIM],
                    in_=feat_v[g][:, : gs[g] * FEAT_DIM],
                ).then_inc(dma_ft[g % FS], 16)
            # outputs
            sync.wait_ge(dve_fin, 2)
            sync.dma_start(out=out_c[:], in_=newc[:]).then_inc(dma_out, 16)
            sync.wait_ge(dve_fin, 3)
            sync.dma_start(out=out_p[:], in_=partials[:]).then_inc(dma_out, 16)
            sync.wait_ge(dma_out, 32)

        @block.vector
        def _(vector):
            vector.memset(counts_acc[:], 0.0)
            vector.memset(ones_f[:], 1.0)
            vector.drain()
            vector.tensor_copy(ones_s[:], ones_f[:])
            vector.drain()
            vector.wait_ge(dma_const, 48)
            for g in range(nG):
                if g >= OS:
                    vector.wait_ge(pe_grp, g - OS + 1)
                w = gs[g]
                lab_b = lab_s[:, offs[g] : offs[g] + w].to_broadcast([P, w, P])
                iota_bg = bass.AP(
                    tensor=iota_f, offset=0,
                    ap=[iota_f.ap().ap[0], [0, w], iota_f.ap().ap[1]],
                )
                vector.tensor_tensor(
                    out=oh[:, g % OS, : w * P].rearrange("p (u c) -> p u c", u=w),
                    in0=iota_bg, in1=lab_b, op=mybir.AluOpType.is_equal,
                ).then_inc(dve_oh, 1)
                vector.wait_ge(dve_oh, g + 1)
                vector.tensor_tensor(
                    out=counts_acc[:, : w * P], in0=counts_acc[:, : w * P],
                    in1=oh[:, g % OS, : w * P],
                    op=mybir.AluOpType.add,
                )
            # counts: reduce u, convert for the final matmul
            vector.drain()
            vector.tensor_reduce(
                counts_pc[:], acc_cu, axis=mybir.AxisListType.X,
                op=mybir.AluOpType.add,
            )
            vector.drain()
            vector.tensor_copy(counts_pc_m[:], counts_pc[:])
            vector.drain().then_inc(dve_fin, 1)
            # blend (needs final sums + counts matmul)
            vector.wait_ge(pe_grp, nG)
            vector.wait_ge(pe_fin, 1)
            vector.tensor_copy(counts_f[:], counts_ps[:, 0:1])
            vector.drain()
            vector.tensor_scalar_max(recip[:], counts_f[:], 1.0)
            vector.tensor_scalar(
                mask[:], counts_f[:], 0.0, None, mybir.AluOpType.is_gt
            )
            vector.drain()
            vector.reciprocal(recip[:], recip[:])
            vector.drain()
            # newc = cent*(1 - a*m) + sums * (recip*m*a)
            vector.tensor_scalar(
                diff[:, 0:1], recip[:], mask[:, :1], ALPHA,
                mybir.AluOpType.mult, mybir.AluOpType.mult,
            )
            vector.tensor_scalar(
                diff[:, 1:2], mask[:], -ALPHA, 1.0,
                mybir.AluOpType.mult, mybir.AluOpType.add,
            )
            vector.drain()
            vector.tensor_scalar_mul(newc[:], cent_s[:], diff[:, 1:2])
            vector.tensor_scalar_mul(mean[:], sums_ps[:], diff[:, 0:1])
            vector.drain()
            vector.tensor_tensor(
                out=newc[:], in0=newc[:], in1=mean[:], op=mybir.AluOpType.add
            )
            vector.drain().then_inc(dve_fin, 1)
            # loss partials
            vector.wait_ge(act_grp, nG + 1)  # csq + all squares done
            vector.tensor_reduce(
                partials[:, 0:1], ssq_cols[:], axis=mybir.AxisListType.X,
                op=mybir.AluOpType.add,
            )
            vector.tensor_tensor(
                out=scr[:], in0=sums_ps[:], in1=cent_s[:], op=mybir.AluOpType.mult
            )
            vector.drain()
            vector.tensor_reduce(
                partials[:, 1:2], scr[:], axis=mybir.AxisListType.X,
                op=mybir.AluOpType.add,
            )
            vector.tensor_tensor(
                out=partials[:, 2:3], in0=csq[:], in1=counts_f[:],
                op=mybir.AluOpType.mult,
            )
            vector.memset(partials[:, 3:4], 0.0)
            vector.drain()
            vector.nop().then_inc(dve_fin, 1)

        @block.tensor
        def _(tensor):
            for g in range(nG):
                tensor.wait_ge(dma_ft[g % FS], 16 * (g // FS + 1))
                tensor.wait_ge(dve_oh, g + 1)
                for u in range(gs[g]):
                    t = offs[g] + u
                    mm = nc.tensor.matmul(
                        sums_ps[:],
                        lhsT=oh[:, g % OS, u * P : (u + 1) * P],
                        rhs=ft[:, g % FS, u * FEAT_DIM : (u + 1) * FEAT_DIM],
                        start=(t == 0), stop=(t == T - 1),
                    )
                    if u == gs[g] - 1:
                        mm.then_inc(pe_grp, 1)
            tensor.wait_ge(dve_fin, 1)
            nc.tensor.matmul(
                counts_ps[:], lhsT=counts_pc_m[:], rhs=ones_s[:],
                start=True, stop=True,
            ).then_inc(pe_fin, 1)

        @block.scalar
        def _(scalar):
            scalar.wait_ge(dma_const, 48)
            nc.scalar.activation(
                sq_scr[:, 1, :FEAT_DIM], cent_s[:],
                mybir.ActivationFunctionType.Square,
                accum_out=csq[:],
            )
            scalar.drain().then_inc(act_grp, 1)
            for g in range(nG):
                scalar.wait_ge(dma_ft[g % FS], 16 * (g // FS + 1))
                nc.scalar.activation(
                    sq_scr[:, g % 2, : gs[g] * FEAT_DIM],
                    ft[:, g % FS, : gs[g] * FEAT_DIM],
                    mybir.ActivationFunctionType.Square,
                    accum_out=ssq_cols[:, g : g + 1],
                ).then_inc(act_grp, 1)

    nc.compile()
    return nc


def _build(T: int) -> bass.Bass:
    """One SPMD program; all 8 cores run it on their own shard."""
    f32 = mybir.dt.float32
    mdt = _mm_dt()
    nG = T // G
    nc = bacc.Bacc(None, target_bir_lowering=False)

    feat = nc.declare_dram_parameter("feat", [nG * P, G * FEAT_DIM], mdt, isOutput=False)
    lab2d = nc.declare_dram_parameter("lab2d", [P, T], f32, isOutput=False)
    cent = nc.declare_dram_parameter("cent", [P, FEAT_DIM], f32, isOutput=False)
    out_c = nc.declare_dram_parameter("out_centers", [P, FEAT_DIM], f32, isOutput=True)
    out_p = nc.declare_dram_parameter("out_partials", [P, 4], f32, isOutput=True)

    with tile.TileContext(nc) as tc:
        with (
            tc.tile_pool(name="singles", bufs=1) as singles,
            tc.tile_pool(name="ftiles", bufs=3) as ftiles,
            tc.tile_pool(name="ohpool", bufs=4) as ohpool,
            tc.tile_pool(name="sqpool", bufs=2) as sqpool,
            tc.tile_pool(name="psum", bufs=1, space="PSUM") as psum,
        ):
            # constants / whole-kernel inputs
            iota_i = singles.tile([P, P], mybir.dt.int32)
            nc.gpsimd.iota(iota_i[:], [[1, P]], channel_multiplier=0)
            iota_f = singles.tile([P, P], f32)
            nc.vector.tensor_copy(iota_f[:], iota_i[:])

            lab_s = singles.tile([P, T], f32)
            nc.sync.dma_start(out=lab_s[:], in_=lab2d[:])
            cent_s = singles.tile([P, FEAT_DIM], f32)
            nc.sync.dma_start(out=cent_s[:], in_=cent[:])
            # N=2 keeps the counts matmul legal under fp32r (even free count)
            ones_f = singles.tile([P, 2], f32)
            nc.vector.memset(ones_f[:], 1.0)
            ones_s = singles.tile([P, 2], mdt)
            nc.vector.tensor_copy(ones_s[:], ones_f[:])
            ssq_cols = singles.tile([P, nG], f32)

            # per-(partition, u, class) one-hot accumulator; reduced to counts
            # at the end (replaces a per-subtile counts matmul)
            counts_acc = singles.tile([P, G * P], f32)
            nc.vector.memset(counts_acc[:], 0.0)

            sums_ps = psum.tile([P, FEAT_DIM], f32)   # one full PSUM bank
            counts_ps = psum.tile([P, 2], f32)        # its own bank (padded)

            feat_v = feat[:].rearrange("(g p) m -> g p m", p=P)
            iota_ap = iota_f[:]
            iota_b = bass.AP(
                tensor=iota_ap.tensor, offset=iota_ap.offset,
                ap=[iota_ap.ap[0], [0, G], iota_ap.ap[1]],
            )  # [P, G, P], same iota row for every u
            for g in range(nG):
                ft = ftiles.tile([P, G, FEAT_DIM], mdt)
                nc.sync.dma_start(out=ft[:], in_=feat_v[g])
                # one wide one-hot for the whole group: oh[p, u, c] = (c == lab[g*G+u][p])
                oh = ohpool.tile([P, G, P], mdt)
                lab_b = lab_s[:, g * G : (g + 1) * G].to_broadcast([P, G, P])
                nc.vector.tensor_tensor(
                    out=oh[:], in0=iota_b, in1=lab_b, op=mybir.AluOpType.is_equal
                )
                nc.vector.tensor_tensor(
                    out=counts_acc[:], in0=counts_acc[:], in1=oh[:].rearrange("p u c -> p (u c)"),
                    op=mybir.AluOpType.add,
                )
                for u in range(G):
                    t = g * G + u
                    nc.tensor.matmul(
                        sums_ps[:], lhsT=oh[:, u, :], rhs=ft[:, u, :],
                        start=(t == 0), stop=(t == T - 1),
                    )
                sq = sqpool.tile([P, G, FEAT_DIM], f32)
                nc.scalar.activation(
                    sq[:], ft[:], mybir.ActivationFunctionType.Square,
                    accum_out=ssq_cols[:, g : g + 1],
                )

            # ---- epilogue: center blend + loss partials ----
            # counts[c] = sum_{p,u} counts_acc[p, u*P + c]: reduce u on DVE,
            # then one matmul with a ones rhs for the partition reduction.
            acc_ap = counts_acc[:]
            acc_cu = bass.AP(
                tensor=acc_ap.tensor, offset=acc_ap.offset,
                ap=[acc_ap.ap[0], [1, P], [P, G]],
            )  # [P, c, u] with u innermost
            counts_pc = singles.tile([P, P], f32)
            nc.vector.tensor_reduce(
                counts_pc[:], acc_cu, axis=mybir.AxisListType.X,
                op=mybir.AluOpType.add,
            )
            counts_pc_m = singles.tile([P, P], mdt)
            nc.vector.tensor_copy(counts_pc_m[:], counts_pc[:])
            nc.tensor.matmul(
                counts_ps[:], lhsT=counts_pc_m[:], rhs=ones_s[:],
                start=True, stop=True,
            )
            counts_f = singles.tile([P, 1], f32)
            nc.vector.tensor_copy(counts_f[:], counts_ps[:, 0:1])
            safe = singles.tile([P, 1], f32)
            nc.vector.tensor_scalar_max(safe[:], counts_f[:], 1.0)
            recip = singles.tile([P, 1], f32)
            nc.vector.reciprocal(recip[:], safe[:])
            mean = singles.tile([P, FEAT_DIM], f32)
            nc.vector.tensor_scalar_mul(mean[:], sums_ps[:], recip[:, :1])
            mask = singles.tile([P, 1], f32)
            nc.vector.tensor_scalar(
                mask[:], counts_f[:], 0.0, None, mybir.AluOpType.is_gt
            )
            diff = singles.tile([P, FEAT_DIM], f32)
            nc.vector.tensor_tensor(
                out=diff[:], in0=mean[:], in1=cent_s[:], op=mybir.AluOpType.subtract
            )
            dm = singles.tile([P, FEAT_DIM], f32)
            # dm = (diff * mask) * ALPHA
            nc.vector.tensor_scalar(
                dm[:], diff[:], mask[:, :1], ALPHA,
                mybir.AluOpType.mult, mybir.AluOpType.mult,
            )
            newc = singles.tile([P, FEAT_DIM], f32)
            nc.vector.tensor_tensor(
                out=newc[:], in0=cent_s[:], in1=dm[:], op=mybir.AluOpType.add
            )
            nc.sync.dma_start(out=out_c[:], in_=newc[:])

            partials = singles.tile([P, 4], f32)
            nc.vector.tensor_reduce(
                partials[:, 0:1], ssq_cols[:], axis=mybir.AxisListType.X,
                op=mybir.AluOpType.add,
            )
            # (tensor_tensor_reduce crashes the exec unit on this HW path;
            # use separate mul + reduce)
            scr = singles.tile([P, FEAT_DIM], f32)
            nc.vector.tensor_tensor(
                out=scr[:], in0=sums_ps[:], in1=cent_s[:], op=mybir.AluOpType.mult
            )
            nc.vector.tensor_reduce(
                partials[:, 1:2], scr[:], axis=mybir.AxisListType.X,
                op=mybir.AluOpType.add,
            )
            scr2 = singles.tile([P, FEAT_DIM], f32)
            csq = singles.tile([P, 1], f32)
            nc.scalar.activation(
                scr2[:], cent_s[:], mybir.ActivationFunctionType.Square,
                accum_out=csq[:],
            )
            nc.vector.tensor_tensor(
                out=partials[:, 2:3], in0=csq[:], in1=counts_f[:],
                op=mybir.AluOpType.mult,
            )
            nc.vector.memset(partials[:, 3:4], 0.0)
            nc.sync.dma_start(out=out_p[:], in_=partials[:])

    nc.compile()
    return nc


def _get_nc(T: int) -> bass.Bass:
    key = (IMPL, MM_DTYPE, T)
    if key not in _NC_CACHE:
        _NC_CACHE[key] = _build_raw(T) if IMPL == "raw" else _build(T)
    return _NC_CACHE[key]


def _shard_inputs(features, labels, centers):
    """Route rows to the core owning their class; build per-core maps."""
    order = np.argsort(labels, kind="stable")
    sl = labels[order]
    bounds = np.searchsorted(sl, np.arange(0, NUM_CLASSES + 1, CPC))
    n_max = int(np.diff(bounds).max())
    T = max(G, math.ceil(n_max / P))
    T = math.ceil(T / G) * G
    Bp = T * P
    nG = T // G

    fdt = _np_feat_dt()
    in_maps = []
    for j in range(N_CORES):
        idx = order[bounds[j] : bounds[j + 1]]
        n = len(idx)
        fj = np.zeros((Bp, FEAT_DIM), fdt)
        fj[:n] = features[idx].astype(fdt)
        lj = np.full(Bp, P - 1, np.float32)  # pad rows -> dummy class 127
        lj[:n] = (sl[bounds[j] : bounds[j + 1]] - j * CPC).astype(np.float32)
        fdev = (
            _pack_features(fj, T)
            if IMPL == "raw"
            else np.ascontiguousarray(
                fj.reshape(nG, G, P, FEAT_DIM).transpose(0, 2, 1, 3)
            ).reshape(nG * P, G * FEAT_DIM)
        )
        ldev = np.ascontiguousarray(lj.reshape(T, P).T)
        cj = np.zeros((P, FEAT_DIM), np.float32)
        cj[:CPC] = centers[j * CPC : (j + 1) * CPC]
        m = {"feat": fdev, "lab2d": ldev, "cent": cj}
        if IMPL == "raw":
            m["iota"] = np.broadcast_to(
                np.arange(P, dtype=np.float32), (P, P)
            ).copy()
        in_maps.append(m)
    return in_maps, T


def _run(features, labels, centers, trace=False):
    in_maps, T = _shard_inputs(features, labels, centers)
    nc = _get_nc(T)
    out = run_bass_kernel_spmd(
        nc, in_maps, core_ids=list(range(N_CORES)), trace=trace
    )
    res = out.results
    new_centers = np.empty((NUM_CLASSES, FEAT_DIM), np.float32)
    ssq = dot = cn = 0.0
    for j in range(N_CORES):
        new_centers[j * CPC : (j + 1) * CPC] = res[j]["out_centers"][:CPC]
        pr = np.asarray(res[j]["out_partials"], dtype=np.float64)
        ssq += pr[:, 0].sum()
        dot += pr[:, 1].sum()
        cn += pr[:, 2].sum()
    B = features.shape[0]
    loss = np.float32(0.5 * (ssq - 2.0 * dot + cn) / B)
    return (loss, new_centers), out


def kernel(features, labels, centers):
    features = np.ascontiguousarray(np.asarray(features), dtype=np.float32)
    labels = np.asarray(labels).astype(np.int64)
    centers = np.ascontiguousarray(np.asarray(centers), dtype=np.float32)
    (loss, new_centers), _ = _run(features, labels, centers, trace=False)
    return loss, new_centers


# revision 37
# speedup vs baseline: 1.3250x; 1.0003x over previous
"""CenterLoss kernel for Trainium2 (8 NeuronCores, Bass/Tile).

Problem (fixed shapes):
    features [32768, 512] f32, labels [32768] int64 in [0, 1000), centers [1000, 512] f32
    loss        = 0.5 * sum((features - centers[labels])^2) / B
    new_centers = where(count>0, (1-a)*centers + a*(segsum(features)/count), centers), a=0.5

Strategy
--------
Shard by CLASS RANGE: core j owns classes [125j, 125j+125). Host routes each
batch row to the core owning its label (this is the shard step — a host-side
fancy-index, same as any data-parallel slice). Each core then scatter-adds its
~4096 rows into ONE 128-partition accumulator tile via a one-hot matmul
(onehot[b, c].T @ feat[b, d], accumulated in PSUM across row tiles), so the PE
does 8x less work than a 1000-class one-hot and no cross-core reduction of the
[1000, 512] state is needed -- classes are owned exclusively.

The gather (centers[labels]) is eliminated algebraically:
    sum_i ||f_i - c_{l_i}||^2 = sum||f||^2 - 2*<segsum(f), centers> + sum_c count_c*||c_c||^2
All three terms come from the segment sums/counts we need anyway.

Everything on device is fp32 (matmul operands bitcast to float32r: exact for
the 0/1 one-hot weights; moving operand passes through the PE fp32 path at
1 cycle/row for N>=256). PSUM accumulation is fp32.

Per-core engine budget (T = 36 row-tiles of 128):
    DMA  ~9 MB in (features) -> ~25 us  <- bound (target_regime=memory)
    PE   36 x (512-cycle sums MM + 1-col counts MM) ~ 10-14 us
    DVE  36 x 128-cycle one-hot is_equal + epilogue ~ 8 us
    ACT  36 x 512-cycle square+row-accum (for sum||f||^2) ~ 13 us
"""

import math
import os

import numpy as np

import concourse.bass as bass
import concourse.bacc as bacc
import concourse.mybir as mybir
import concourse.tile as tile
from concourse.bass_utils import run_bass_kernel_spmd

NUM_CLASSES = 1000
FEAT_DIM = 512
ALPHA = 0.5
N_CORES = 8
CPC = NUM_CLASSES // N_CORES  # 125 classes per core
P = 128
G = int(os.environ.get("CENTERLOSS_G", "6"))  # 128-row subtiles per DMA group

# matmul operand mode:
#   "f32r" - PE fast fp32 path (1 cyc/row at N>=256), DMA stays 4B/elem
#   "f32"  - exact fp32, 4 cyc/row on PE
#   "bf16" - operands in bf16: halves feature DMA traffic; PSUM accum fp32
MM_DTYPE = os.environ.get("CENTERLOSS_MM_DTYPE", "f32r")
# "raw" = hand-scheduled engine blocks; "tile" = TileContext version
IMPL = os.environ.get("CENTERLOSS_IMPL", "raw")

_NC_CACHE: dict = {}


def _mm_dt():
    return {
        "f32r": mybir.dt.float32r,
        "f32": mybir.dt.float32,
        "bf16": mybir.dt.bfloat16,
    }[MM_DTYPE]


def _np_feat_dt():
    import ml_dtypes

    return ml_dtypes.bfloat16 if MM_DTYPE == "bf16" else np.float32


def _group_sizes(T: int):
    return [G] * (T // G)


def _pack_features(fj, T):
    """Pack [Bp, D] rows into the device layout: one [P, w*D] block per group
    (subtile u of group g -> block cols [u*D, (u+1)*D)), zero-padded to G*D."""
    gs = _group_sizes(T)
    nG = len(gs)
    out = np.zeros((nG * P, G * FEAT_DIM), fj.dtype)
    off = 0
    for k, w in enumerate(gs):
        blk = fj[off * P : (off + w) * P].reshape(w, P, FEAT_DIM)
        out[k * P : (k + 1) * P, : w * FEAT_DIM] = (
            blk.transpose(1, 0, 2).reshape(P, w * FEAT_DIM)
        )
        off += w
    return out


def _build_raw(T: int) -> bass.Bass:
    """Raw-bacc pipeline (no TileContext): explicit engine programs and
    semaphores. Avoids Tile's ~3us start barrier and ~13us drain butterfly.

    Engines: Sync issues DMAs (triple-buffered feature groups), Vector builds
    the wide one-hot + counts accumulator + epilogue blend, Tensor runs the
    PSUM-accumulated one-hot matmuls, Scalar does square+row-accumulate.
    """
    f32 = mybir.dt.float32
    mdt = _mm_dt()
    gs = _group_sizes(T)
    nG = len(gs)
    offs = [sum(gs[:k]) for k in range(nG)]
    FS = 3  # ft slots
    OS = 2  # oh slots
    nc = bacc.Bacc(None, target_bir_lowering=False)

    feat = nc.declare_dram_parameter("feat", [nG * P, G * FEAT_DIM], mdt, isOutput=False)
    lab2d = nc.declare_dram_parameter("lab2d", [P, T], f32, isOutput=False)
    cent = nc.declare_dram_parameter("cent", [P, FEAT_DIM], f32, isOutput=False)
    iota_in = nc.declare_dram_parameter("iota", [P, P], f32, isOutput=False)
    out_c = nc.declare_dram_parameter("out_centers", [P, FEAT_DIM], f32, isOutput=True)
    out_p = nc.declare_dram_parameter("out_partials", [P, 4], f32, isOutput=True)

    feat_v = feat[:].rearrange("(g p) m -> g p m", p=P)

    from contextlib import ExitStack

    ctx = ExitStack()
    with ctx:
        ft = ctx.enter_context(nc.sbuf_tensor("ft", [P, FS, G * FEAT_DIM], mdt))
        oh = ctx.enter_context(nc.sbuf_tensor("oh", [P, OS, G * P], mdt))
        iota_f = ctx.enter_context(nc.sbuf_tensor("iota_f", [P, P], f32))
        lab_s = ctx.enter_context(nc.sbuf_tensor("lab_s", [P, T], f32))
        cent_s = ctx.enter_context(nc.sbuf_tensor("cent_s", [P, FEAT_DIM], f32))
        counts_acc = ctx.enter_context(nc.sbuf_tensor("counts_acc", [P, G * P], f32))
        ssq_cols = ctx.enter_context(nc.sbuf_tensor("ssq_cols", [P, nG], f32))
        sq_scr = ctx.enter_context(nc.sbuf_tensor("sq_scr", [P, 2, G * FEAT_DIM], f32))
        ones_s = ctx.enter_context(nc.sbuf_tensor("ones_s", [P, 2], mdt))
        ones_f = ctx.enter_context(nc.sbuf_tensor("ones_f", [P, 2], f32))
        counts_pc = ctx.enter_context(nc.sbuf_tensor("counts_pc", [P, P], f32))
        counts_pc_m = ctx.enter_context(nc.sbuf_tensor("counts_pc_m", [P, P], mdt))
        counts_f = ctx.enter_context(nc.sbuf_tensor("counts_f", [P, 1], f32))
        recip = ctx.enter_context(nc.sbuf_tensor("recip", [P, 1], f32))
        mask = ctx.enter_context(nc.sbuf_tensor("mask", [P, 1], f32))
        mean = ctx.enter_context(nc.sbuf_tensor("mean", [P, FEAT_DIM], f32))
        diff = ctx.enter_context(nc.sbuf_tensor("diff", [P, FEAT_DIM], f32))
        newc = ctx.enter_context(nc.sbuf_tensor("newc", [P, FEAT_DIM], f32))
        scr = ctx.enter_context(nc.sbuf_tensor("scr", [P, FEAT_DIM], f32))
        csq = ctx.enter_context(nc.sbuf_tensor("csq", [P, 1], f32))
        partials = ctx.enter_context(nc.sbuf_tensor("partials", [P, 4], f32))
        sums_ps = ctx.enter_context(nc.psum_tensor("sums_ps", [P, FEAT_DIM], f32))
        counts_ps = ctx.enter_context(nc.psum_tensor("counts_ps", [P, 2], f32))
        dma_const = ctx.enter_context(nc.semaphore("dma_const"))
        dma_ft = [
            ctx.enter_context(nc.semaphore(f"dma_ft{i}")) for i in range(FS)
        ]
        dve_oh = ctx.enter_context(nc.semaphore("dve_oh"))
        pe_grp = ctx.enter_context(nc.semaphore("pe_grp"))
        act_grp = ctx.enter_context(nc.semaphore("act_grp"))
        dve_fin = ctx.enter_context(nc.semaphore("dve_fin"))
        pe_fin = ctx.enter_context(nc.semaphore("pe_fin"))
        dma_out = ctx.enter_context(nc.semaphore("dma_out"))
        block = ctx.enter_context(nc.Block())
        iota_b = bass.AP(
            tensor=iota_f, offset=0,
            ap=[iota_f.ap().ap[0], [0, G], iota_f.ap().ap[1]],
        )  # [P, G, P]
        acc_cu = bass.AP(
            tensor=counts_acc, offset=0,
            ap=[counts_acc.ap().ap[0], [1, P], [P, G]],
        )  # [P, c, u]

        @block.sync
        def _(sync):
            # first feature group leads; tiny constants right behind it
            sync.dma_start(
                out=ft[:, 0, : gs[0] * FEAT_DIM],
                in_=feat_v[0][:, : gs[0] * FEAT_DIM],
            ).then_inc(dma_ft[0], 16)
            sync.dma_start(out=lab_s[:], in_=lab2d[:]).then_inc(dma_const, 16)
            sync.dma_start(out=iota_f[:], in_=iota_in[:]).then_inc(dma_const, 16)
            sync.dma_start(out=cent_s[:], in_=cent[:]).then_inc(dma_const, 16)
            # (ft0 was issued first above; remaining groups below)
            for g in range(1, nG):
                if g >= FS:
                    # slot free when group g-FS fully consumed by PE and ACT
                    sync.wait_ge(pe_grp, g - FS + 1)
                    sync.wait_ge(act_grp, g - FS + 2)
                sync.dma_start(
                    out=ft[:, g % FS, : gs[g] * FEAT_DIM],
                    in_=feat_v[g][:, : gs[g] * FEAT_DIM],
                ).then_inc(dma_ft[g % FS], 16)
            # outputs
            sync.wait_ge(dve_fin, 2)
            sync.dma_start(out=out_c[:], in_=newc[:]).then_inc(dma_out, 16)
            sync.wait_ge(dve_fin, 3)
            sync.dma_start(out=out_p[:], in_=partials[:]).then_inc(dma_out, 16)
            sync.wait_ge(dma_out, 32)

        @block.vector
        def _(vector):
            vector.memset(counts_acc[:], 0.0)
            vector.memset(ones_f[:], 1.0)
            vector.drain()
            vector.tensor_copy(ones_s[:], ones_f[:])
            vector.drain()
            vector.wait_ge(dma_const, 48)
            for g in range(nG):
                if g >= OS:
                    vector.wait_ge(pe_grp, g - OS + 1)
                w = gs[g]
                lab_b = lab_s[:, offs[g] : offs[g] + w].to_broadcast([P, w, P])
                iota_bg = bass.AP(
                    tensor=iota_f, offset=0,
                    ap=[iota_f.ap().ap[0], [0, w], iota_f.ap().ap[1]],
                )
                vector.tensor_tensor(
                    out=oh[:, g % OS, : w * P].rearrange("p (u c) -> p u c", u=w),
                    in0=iota_bg, in1=lab_b, op=mybir.AluOpType.is_equal,
                ).then_inc(dve_oh, 1)
                vector.wait_ge(dve_oh, g + 1)
                vector.tensor_tensor(
                    out=counts_acc[:, : w * P], in0=counts_acc[:, : w * P],
                    in1=oh[:, g % OS, : w * P],
                    op=mybir.AluOpType.add,
                )
            # counts: reduce u, convert for the final matmul
            vector.drain()
            vector.tensor_reduce(
                counts_pc[:], acc_cu, axis=mybir.AxisListType.X,
                op=mybir.AluOpType.add,
            )
            vector.drain()
            vector.tensor_copy(counts_pc_m[:], counts_pc[:])
            vector.drain().then_inc(dve_fin, 1)
            # blend (needs final sums + counts matmul)
            vector.wait_ge(pe_grp, nG)
            vector.wait_ge(pe_fin, 1)
            vector.tensor_copy(counts_f[:], counts_ps[:, 0:1])
            vector.drain()
            vector.tensor_scalar_max(recip[:], counts_f[:], 1.0)
            vector.tensor_scalar(
                mask[:], counts_f[:], 0.0, None, mybir.AluOpType.is_gt
            )
            vector.drain()
            vector.reciprocal(recip[:], recip[:])
            vector.drain()
            # newc = cent*(1 - a*m) + sums * (recip*m*a)
            vector.tensor_scalar(
                diff[:, 0:1], recip[:], mask[:, :1], ALPHA,
                mybir.AluOpType.mult, mybir.AluOpType.mult,
            )
            vector.tensor_scalar(
                diff[:, 1:2], mask[:], -ALPHA, 1.0,
                mybir.AluOpType.mult, mybir.AluOpType.add,
            )
            vector.drain()
            vector.tensor_scalar_mul(newc[:], cent_s[:], diff[:, 1:2])
            vector.tensor_scalar_mul(mean[:], sums_ps[:], diff[:, 0:1])
            vector.drain()
            vector.tensor_tensor(
                out=newc[:], in0=newc[:], in1=mean[:], op=mybir.AluOpType.add
            )
            vector.drain().then_inc(dve_fin, 1)
            # loss partials
            vector.wait_ge(act_grp, nG + 1)  # csq + all squares done
            vector.tensor_reduce(
                partials[:, 0:1], ssq_cols[:], axis=mybir.AxisListType.X,
                op=mybir.AluOpType.add,
            )
            vector.tensor_tensor(
                out=scr[:], in0=sums_ps[:], in1=cent_s[:], op=mybir.AluOpType.mult
            )
            vector.drain()
            vector.tensor_reduce(
                partials[:, 1:2], scr[:], axis=mybir.AxisListType.X,
                op=mybir.AluOpType.add,
            )
            vector.tensor_tensor(
                out=partials[:, 2:3], in0=csq[:], in1=counts_f[:],
                op=mybir.AluOpType.mult,
            )
            vector.memset(partials[:, 3:4], 0.0)
            vector.drain()
            vector.nop().then_inc(dve_fin, 1)

        @block.tensor
        def _(tensor):
            for g in range(nG):
                tensor.wait_ge(dma_ft[g % FS], 16 * (g // FS + 1))
                tensor.wait_ge(dve_oh, g + 1)
                for u in range(gs[g]):
                    t = offs[g] + u
                    mm = nc.tensor.matmul(
                        sums_ps[:],
                        lhsT=oh[:, g % OS, u * P : (u + 1) * P],
                        rhs=ft[:, g % FS, u * FEAT_DIM : (u + 1) * FEAT_DIM],
                        start=(t == 0), stop=(t == T - 1),
                    )
                    if u == gs[g] - 1:
                        mm.then_inc(pe_grp, 1)
            tensor.wait_ge(dve_fin, 1)
            nc.tensor.matmul(
                counts_ps[:], lhsT=counts_pc_m[:], rhs=ones_s[:],
                start=True, stop=True,
            ).then_inc(pe_fin, 1)

        @block.scalar
        def _(scalar):
            scalar.wait_ge(dma_const, 48)
            nc.scalar.activation(
                sq_scr[:, 1, :FEAT_DIM], cent_s[:],
                mybir.ActivationFunctionType.Square,
                accum_out=csq[:],
            )
            scalar.drain().then_inc(act_grp, 1)
            for g in range(nG):
                scalar.wait_ge(dma_ft[g % FS], 16 * (g // FS + 1))
                nc.scalar.activation(
                    sq_scr[:, g % 2, : gs[g] * FEAT_DIM],
                    ft[:, g % FS, : gs[g] * FEAT_DIM],
                    mybir.ActivationFunctionType.Square,
                    accum_out=ssq_cols[:, g : g + 1],
                ).then_inc(act_grp, 1)

    nc.compile()
    return nc


def _build(T: int) -> bass.Bass:
    """One SPMD program; all 8 cores run it on their own shard."""
    f32 = mybir.dt.float32
    mdt = _mm_dt()
    nG = T // G
    nc = bacc.Bacc(None, target_bir_lowering=False)

    feat = nc.declare_dram_parameter("feat", [nG * P, G * FEAT_DIM], mdt, isOutput=False)
    lab2d = nc.declare_dram_parameter("lab2d", [P, T], f32, isOutput=False)
    cent = nc.declare_dram_parameter("cent", [P, FEAT_DIM], f32, isOutput=False)
    out_c = nc.declare_dram_parameter("out_centers", [P, FEAT_DIM], f32, isOutput=True)
    out_p = nc.declare_dram_parameter("out_partials", [P, 4], f32, isOutput=True)

    with tile.TileContext(nc) as tc:
        with (
            tc.tile_pool(name="singles", bufs=1) as singles,
            tc.tile_pool(name="ftiles", bufs=3) as ftiles,
            tc.tile_pool(name="ohpool", bufs=4) as ohpool,
            tc.tile_pool(name="sqpool", bufs=2) as sqpool,
            tc.tile_pool(name="psum", bufs=1, space="PSUM") as psum,
        ):
            # constants / whole-kernel inputs
            iota_i = singles.tile([P, P], mybir.dt.int32)
            nc.gpsimd.iota(iota_i[:], [[1, P]], channel_multiplier=0)
            iota_f = singles.tile([P, P], f32)
            nc.vector.tensor_copy(iota_f[:], iota_i[:])

            lab_s = singles.tile([P, T], f32)
            nc.sync.dma_start(out=lab_s[:], in_=lab2d[:])
            cent_s = singles.tile([P, FEAT_DIM], f32)
            nc.sync.dma_start(out=cent_s[:], in_=cent[:])
            # N=2 keeps the counts matmul legal under fp32r (even free count)
            ones_f = singles.tile([P, 2], f32)
            nc.vector.memset(ones_f[:], 1.0)
            ones_s = singles.tile([P, 2], mdt)
            nc.vector.tensor_copy(ones_s[:], ones_f[:])
            ssq_cols = singles.tile([P, nG], f32)

            # per-(partition, u, class) one-hot accumulator; reduced to counts
            # at the end (replaces a per-subtile counts matmul)
            counts_acc = singles.tile([P, G * P], f32)
            nc.vector.memset(counts_acc[:], 0.0)

            sums_ps = psum.tile([P, FEAT_DIM], f32)   # one full PSUM bank
            counts_ps = psum.tile([P, 2], f32)        # its own bank (padded)

            feat_v = feat[:].rearrange("(g p) m -> g p m", p=P)
            iota_ap = iota_f[:]
            iota_b = bass.AP(
                tensor=iota_ap.tensor, offset=iota_ap.offset,
                ap=[iota_ap.ap[0], [0, G], iota_ap.ap[1]],
            )  # [P, G, P], same iota row for every u
            for g in range(nG):
                ft = ftiles.tile([P, G, FEAT_DIM], mdt)
                nc.sync.dma_start(out=ft[:], in_=feat_v[g])
                # one wide one-hot for the whole group: oh[p, u, c] = (c == lab[g*G+u][p])
                oh = ohpool.tile([P, G, P], mdt)
                lab_b = lab_s[:, g * G : (g + 1) * G].to_broadcast([P, G, P])
                nc.vector.tensor_tensor(
                    out=oh[:], in0=iota_b, in1=lab_b, op=mybir.AluOpType.is_equal
                )
                nc.vector.tensor_tensor(
                    out=counts_acc[:], in0=counts_acc[:], in1=oh[:].rearrange("p u c -> p (u c)"),
                    op=mybir.AluOpType.add,
                )
                for u in range(G):
                    t = g * G + u
                    nc.tensor.matmul(
                        sums_ps[:], lhsT=oh[:, u, :], rhs=ft[:, u, :],
                        start=(t == 0), stop=(t == T - 1),
                    )
                sq = sqpool.tile([P, G, FEAT_DIM], f32)
                nc.scalar.activation(
                    sq[:], ft[:], mybir.ActivationFunctionType.Square,
                    accum_out=ssq_cols[:, g : g + 1],
                )

            # ---- epilogue: center blend + loss partials ----
            # counts[c] = sum_{p,u} counts_acc[p, u*P + c]: reduce u on DVE,
            # then one matmul with a ones rhs for the partition reduction.
            acc_ap = counts_acc[:]
            acc_cu = bass.AP(
                tensor=acc_ap.tensor, offset=acc_ap.offset,
                ap=[acc_ap.ap[0], [1, P], [P, G]],
            )  # [P, c, u] with u innermost
            counts_pc = singles.tile([P, P], f32)
            nc.vector.tensor_reduce(
                counts_pc[:], acc_cu, axis=mybir.AxisListType.X,
                op=mybir.AluOpType.add,
            )
            counts_pc_m = singles.tile([P, P], mdt)
            nc.vector.tensor_copy(counts_pc_m[:], counts_pc[:])
            nc.tensor.matmul(
                counts_ps[:], lhsT=counts_pc_m[:], rhs=ones_s[:],
                start=True, stop=True,
            )
            counts_f = singles.tile([P, 1], f32)
            nc.vector.tensor_copy(counts_f[:], counts_ps[:, 0:1])
            safe = singles.tile([P, 1], f32)
            nc.vector.tensor_scalar_max(safe[:], counts_f[:], 1.0)
            recip = singles.tile([P, 1], f32)
            nc.vector.reciprocal(recip[:], safe[:])
            mean = singles.tile([P, FEAT_DIM], f32)
            nc.vector.tensor_scalar_mul(mean[:], sums_ps[:], recip[:, :1])
            mask = singles.tile([P, 1], f32)
            nc.vector.tensor_scalar(
                mask[:], counts_f[:], 0.0, None, mybir.AluOpType.is_gt
            )
            diff = singles.tile([P, FEAT_DIM], f32)
            nc.vector.tensor_tensor(
                out=diff[:], in0=mean[:], in1=cent_s[:], op=mybir.AluOpType.subtract
            )
            dm = singles.tile([P, FEAT_DIM], f32)
            # dm = (diff * mask) * ALPHA
            nc.vector.tensor_scalar(
                dm[:], diff[:], mask[:, :1], ALPHA,
                mybir.AluOpType.mult, mybir.AluOpType.mult,
            )
            newc = singles.tile([P, FEAT_DIM], f32)
            nc.vector.tensor_tensor(
                out=newc[:], in0=cent_s[:], in1=dm[:], op=mybir.AluOpType.add
            )
            nc.sync.dma_start(out=out_c[:], in_=newc[:])

            partials = singles.tile([P, 4], f32)
            nc.vector.tensor_reduce(
                partials[:, 0:1], ssq_cols[:], axis=mybir.AxisListType.X,
                op=mybir.AluOpType.add,
            )
            # (tensor_tensor_reduce crashes the exec unit on this HW path;
            # use separate mul + reduce)
            scr = singles.tile([P, FEAT_DIM], f32)
            nc.vector.tensor_tensor(
                out=scr[:], in0=sums_ps[:], in1=cent_s[:], op=mybir.AluOpType.mult
            )
            nc.vector.tensor_reduce(
                partials[:, 1:2], scr[:], axis=mybir.AxisListType.X,
                op=mybir.AluOpType.add,
            )
            scr2 = singles.tile([P, FEAT_DIM], f32)
            csq = singles.tile([P, 1], f32)
            nc.scalar.activation(
                scr2[:], cent_s[:], mybir.ActivationFunctionType.Square,
                accum_out=csq[:],
            )
            nc.vector.tensor_tensor(
                out=partials[:, 2:3], in0=csq[:], in1=counts_f[:],
                op=mybir.AluOpType.mult,
            )
            nc.vector.memset(partials[:, 3:4], 0.0)
            nc.sync.dma_start(out=out_p[:], in_=partials[:])

    nc.compile()
    return nc


def _get_nc(T: int) -> bass.Bass:
    key = (IMPL, MM_DTYPE, G, T)
    if key not in _NC_CACHE:
        _NC_CACHE[key] = _build_raw(T) if IMPL == "raw" else _build(T)
    return _NC_CACHE[key]


def _shard_inputs(features, labels, centers):
    """Route rows to the core owning their class; build per-core maps."""
    order = np.argsort(labels, kind="stable")
    sl = labels[order]
    bounds = np.searchsorted(sl, np.arange(0, NUM_CLASSES + 1, CPC))
    n_max = int(np.diff(bounds).max())
    T = max(G, math.ceil(n_max / P))
    T = math.ceil(T / G) * G
    Bp = T * P
    nG = T // G

    fdt = _np_feat_dt()
    in_maps = []
    for j in range(N_CORES):
        idx = order[bounds[j] : bounds[j + 1]]
        n = len(idx)
        fj = np.zeros((Bp, FEAT_DIM), fdt)
        fj[:n] = features[idx].astype(fdt)
        lj = np.full(Bp, P - 1, np.float32)  # pad rows -> dummy class 127
        lj[:n] = (sl[bounds[j] : bounds[j + 1]] - j * CPC).astype(np.float32)
        fdev = (
            _pack_features(fj, T)
            if IMPL == "raw"
            else np.ascontiguousarray(
                fj.reshape(nG, G, P, FEAT_DIM).transpose(0, 2, 1, 3)
            ).reshape(nG * P, G * FEAT_DIM)
        )
        ldev = np.ascontiguousarray(lj.reshape(T, P).T)
        cj = np.zeros((P, FEAT_DIM), np.float32)
        cj[:CPC] = centers[j * CPC : (j + 1) * CPC]
        m = {"feat": fdev, "lab2d": ldev, "cent": cj}
        if IMPL == "raw":
            m["iota"] = np.broadcast_to(
                np.arange(P, dtype=np.float32), (P, P)
            ).copy()
        in_maps.append(m)
    return in_maps, T


def _run(features, labels, centers, trace=False):
    in_maps, T = _shard_inputs(features, labels, centers)
    nc = _get_nc(T)
    out = run_bass_kernel_spmd(
        nc, in_maps, core_ids=list(range(N_CORES)), trace=trace
    )
    res = out.results
    new_centers = np.empty((NUM_CLASSES, FEAT_DIM), np.float32)
    ssq = dot = cn = 0.0
    for j in range(N_CORES):
        new_centers[j * CPC : (j + 1) * CPC] = res[j]["out_centers"][:CPC]
        pr = np.asarray(res[j]["out_partials"], dtype=np.float64)
        ssq += pr[:, 0].sum()
        dot += pr[:, 1].sum()
        cn += pr[:, 2].sum()
    B = features.shape[0]
    loss = np.float32(0.5 * (ssq - 2.0 * dot + cn) / B)
    return (loss, new_centers), out


def kernel(features, labels, centers):
    features = np.ascontiguousarray(np.asarray(features), dtype=np.float32)
    labels = np.asarray(labels).astype(np.int64)
    centers = np.ascontiguousarray(np.asarray(centers), dtype=np.float32)
    (loss, new_centers), _ = _run(features, labels, centers, trace=False)
    return loss, new_centers
